# revision 12
# baseline (speedup 1.0000x reference)
"""Trainium2 kernel for nn_Conv_RBS_density (496x496 density-matrix RBS circuit).

The reference applies 48 sequential RBS-gate conjugations
``rho <- U rho U^T`` where every ``U = cos(t)*A + sin(t)*B + C`` is an
orthogonal matrix of 30 disjoint 2x2 Givens rotations.  By associativity
the whole pipeline is ``out = V rho V^T`` with ``V = U48 @ ... @ U1``.
V is accumulated on the host with sparse Givens row updates
(O(48*30*N) flops — negligible); the O(N^3) work — the dense matmuls
against rho — runs on the 8 NeuronCores.

Structure: gates only couple basis states within connected components of
the qubit-tile graph, so V is block-diagonal (28 components of 16 states
+ 8 of 6).  Packing components into 4 bins of exactly 124 states gives a
grouped order where V_g = diag(B0..B3), each 124x124, and
``out_ij = B_i rho_ij B_j^T`` for the 16 124x124 blocks of the output.

Sharding: one (i, j-pair) per core — core k (i = k//2, h = k%2) computes
out blocks (i, 2h) and (i, 2h+1).  Per block: mm1 ``pk = rho_ij^T @ B_i^T``
(= (B_i rho_ij)^T, using rho's symmetry to avoid any transpose), then
mm2 ``out_ij = pk^T @ B_j^T``.  All operands bf16 (tolerance is 2e-2;
bf16 products accumulate in fp32 PSUM so only input rounding ~4e-3
matters) which makes matmuls 1-pass (4x fewer PE cycles than fp32) and
halves DMA bytes.  Only the two HWDGE queues (sync/scalar) are used —
SWDGE (gpsimd) adds ~3us of preamble memsets + postamble drain.

DMA layout: d1 (sync) = [B_i^T | rho_ij0] gates mm1_a as early as
possible; d2 (scalar) = [B_j0^T | rho_ij1 | B_j1^T] carries everything
else; outputs go out fp32 as two DMAs (scalar, then sync) as soon as
each block's PSUM->SBUF copy lands.  PE order mm1_a, mm1_b, mm2_a,
mm2_b keeps the PSUM copies off the critical path.
"""

import numpy as np

import concourse.mybir as mybir
from concourse import bacc
from concourse.bass_utils import run_bass_kernel_spmd
from concourse.tile import TileContext

N = 496          # C(32, 2) Hamming-weight-2 states
NCORES = 8
BK = 124         # packed block size
NB = N // BK     # 4 blocks

_cache = {}


def _build_program():
    nc = bacc.Bacc(
        "TRN2", target_bir_lowering=False, debug=False, num_devices=NCORES
    )
    f32 = mybir.dt.float32
    bf16 = mybir.dt.bfloat16
    d1_d = nc.dram_tensor("d1", [BK, 2 * BK], bf16, kind="ExternalInput")
    d2a_d = nc.dram_tensor("d2a", [BK, 2 * BK], bf16, kind="ExternalInput")
    d2b_d = nc.dram_tensor("d2b", [BK, BK], bf16, kind="ExternalInput")
    oa_d = nc.dram_tensor("oa", [BK, BK], bf16, kind="ExternalOutput")
    ob_d = nc.dram_tensor("ob", [BK, BK], bf16, kind="ExternalOutput")

    with TileContext(nc) as tc:
        with (
            tc.tile_pool(name="sbuf", bufs=1) as sbuf,
            tc.tile_pool(name="psum", bufs=1, space="PSUM") as psum,
        ):
            t1 = sbuf.tile([BK, 2 * BK], bf16, tag="t1", name="t1")
            t2a = sbuf.tile([BK, 2 * BK], bf16, tag="t2a", name="t2a")
            t2b = sbuf.tile([BK, BK], bf16, tag="t2b", name="t2b")
            # d1 and d2a are hoisted into the entry block below (their
            # packet streams interleave on the shared 4-SDMA-engine
            # HWDGE pool and both land ~9.4us); d2b is issued in the
            # body on sync, landing just before mm2_b needs it.  The
            # scalar ring's descriptor generation is ~2x slower but its
            # engine can issue ~0.8us earlier than sync (which has a
            # ~700ns runtime drain first).
            nc.scalar.dma_start(t1[:], d1_d[:, :])
            nc.sync.dma_start(t2a[:], d2a_d[:, :])
            BiT = t1[:, 0:BK]
            rho_a = t1[:, BK : 2 * BK]
            Bj0T = t2a[:, 0:BK]
            rho_b = t2a[:, BK : 2 * BK]
            Bj1T = t2b[:, 0:BK]

            # mm1: pk = (B_i rho_ij)^T chunks, orientation ready for mm2 lhsT
            pka_ps = psum.tile([BK, BK], f32, tag="pka", name="pka")
            nc.tensor.matmul(pka_ps[:], rho_a, BiT, start=True, stop=True)
            pkb_ps = psum.tile([BK, BK], f32, tag="pkb", name="pkb")
            nc.tensor.matmul(pkb_ps[:], rho_b, BiT, start=True, stop=True)
            # issued here (after the matmuls in program order) so the
            # hoist pass below leaves it in the body
            nc.sync.dma_start(t2b[:], d2b_d[:, :])
            pka = sbuf.tile([BK, BK], bf16, tag="pka_sb", name="pka_sb")
            nc.vector.tensor_copy(pka[:], pka_ps[:])
            pkb = sbuf.tile([BK, BK], bf16, tag="pkb_sb", name="pkb_sb")
            nc.vector.tensor_copy(pkb[:], pkb_ps[:])

            # mm2: out_ij = pk^T @ B_j^T, DMA'd out as soon as copied
            oa_ps = psum.tile([BK, BK], f32, tag="oa", name="oa_ps")
            nc.tensor.matmul(oa_ps[:], pka[:], Bj0T, start=True, stop=True)
            oa_sb = sbuf.tile([BK, BK], bf16, tag="oa_sb", name="oa_sb")
            nc.scalar.copy(oa_sb[:], oa_ps[:])
            nc.scalar.dma_start(oa_d[:, :], oa_sb[:])

            ob_ps = psum.tile([BK, BK], f32, tag="ob", name="ob_ps")
            nc.tensor.matmul(ob_ps[:], pkb[:], Bj1T, start=True, stop=True)
            ob_sb = sbuf.tile([BK, BK], bf16, tag="ob_sb", name="ob_sb")
            nc.vector.tensor_copy(ob_sb[:], ob_ps[:])
            nc.sync.dma_start(ob_d[:, :], ob_sb[:])

    # Hoist the input DMAs from the body block into the entry block so
    # their ~1us descriptor generation and ~2us HBM latency overlap the
    # framework's entry barrier instead of following it.  They are
    # placed after the per-engine TPB-base register setup (their APs may
    # reference those base registers) and before the entry barrier; the
    # consumers' semaphore waits stay in the body, so ordering is
    # preserved.  Input DMA completion sems start at zero (the previous
    # NEFF execution's postamble sweep clears sems 7..255).
    entry, body = nc.main_func.blocks[0], nc.main_func.blocks[1]
    moved = []
    for ins in list(body.instructions):
        if isinstance(ins, (mybir.InstMatmult, mybir.InstLdweights)):
            break
        if (
            isinstance(ins, mybir.InstDMACopy)
            and ins.engine == mybir.EngineType.SP
        ):
            moved.append(ins)
    for ins in moved:
        body.instructions.remove(ins)
    pos = next(
        i
        for i, ins in enumerate(entry.instructions)
        if isinstance(ins, mybir.InstMemset)
    )
    for off, ins in enumerate(moved):
        entry.instructions.insert(pos + off, ins)

    nc.compile()
    return nc


def _program():
    if "nc" not in _cache:
        _cache["nc"] = _build_program()
    return _cache["nc"]


def _gate_pairs(B_stack):
    """Per unique gate: (s, q) index arrays with B[u, s, q] = +1."""
    pairs = []
    for u in range(B_stack.shape[0]):
        pos = np.argwhere(B_stack[u] > 0.5)
        pairs.append((pos[:, 0], pos[:, 1]))
    return pairs


def _build_V(thetas, pairs, u_idx, p_idx, n):
    """V = U_G ... U_1 via sparse Givens row updates (float64)."""
    thetas = np.asarray(thetas, np.float64)
    cos_t, sin_t = np.cos(thetas), np.sin(thetas)
    V = np.eye(n)
    for g in range(len(u_idx)):
        u, p = int(u_idx[g]), int(p_idx[g])
        c, s = cos_t[p], sin_t[p]
        S, Q = pairs[u]
        vs, vq = V[S], V[Q]
        V[S] = c * vs + s * vq
        V[Q] = -s * vs + c * vq
    return V


def _grouping(pairs, n):
    """Union states coupled by any gate; pack components into NB bins of BK."""
    parent = list(range(n))

    def find(a):
        while parent[a] != a:
            parent[a] = parent[parent[a]]
            a = parent[a]
        return a

    for S, Q in pairs:
        for s, q in zip(S.tolist(), Q.tolist()):
            ra, rb = find(s), find(q)
            if ra != rb:
                parent[ra] = rb

    comps = {}
    for i in range(n):
        comps.setdefault(find(i), []).append(i)
    comps = sorted(comps.values(), key=len, reverse=True)

    bins = [[] for _ in range(NB)]
    for comp in comps:
        for b in bins:
            if len(b) + len(comp) <= BK:
                b.extend(comp)
                break
        else:
            raise ValueError("component packing failed")
    assert all(len(b) == BK for b in bins), [len(b) for b in bins]
    return np.array([i for b in bins for i in b], np.int64)


def _run(rho, thetas, A_stack, B_stack, C_stack, u_idx, p_idx, trace=False):
    import ml_dtypes

    bf16 = ml_dtypes.bfloat16
    rho = np.asarray(rho, np.float32)
    B_stack = np.asarray(B_stack)
    u_idx = np.asarray(u_idx).astype(np.int64)
    p_idx = np.asarray(p_idx).astype(np.int64)
    n = rho.shape[0]
    assert n == N, n

    if "struct" not in _cache:
        pairs = _gate_pairs(B_stack)
        _cache["struct"] = (pairs, _grouping(pairs, n))
    pairs, perm = _cache["struct"]

    V = _build_V(thetas, pairs, u_idx, p_idx, n).astype(np.float32)
    V_g = V[np.ix_(perm, perm)]
    rho_g = rho[np.ix_(perm, perm)]

    # block-diagonality check (structure is fixed by the module definition)
    blocks = [
        V_g[j * BK : (j + 1) * BK, j * BK : (j + 1) * BK] for j in range(NB)
    ]
    bd = np.zeros_like(V_g)
    for j in range(NB):
        bd[j * BK : (j + 1) * BK, j * BK : (j + 1) * BK] = blocks[j]
    assert np.array_equal(bd, V_g), "V lost block-diagonal structure"

    Bt = [np.ascontiguousarray(b.T).astype(bf16) for b in blocks]
    rho_bf = rho_g.astype(bf16)

    in_maps = []
    for k in range(NCORES):
        i, h = divmod(k, 2)
        j0, j1 = 2 * h, 2 * h + 1
        rr = rho_bf[i * BK : (i + 1) * BK]
        rho_a = rr[:, j0 * BK : (j0 + 1) * BK]
        rho_b = rr[:, j1 * BK : (j1 + 1) * BK]
        d1 = np.ascontiguousarray(np.concatenate([Bt[i], rho_a], axis=1))
        d2a = np.ascontiguousarray(np.concatenate([Bt[j0], rho_b], axis=1))
        d2b = np.ascontiguousarray(Bt[j1])
        in_maps.append({"d1": d1, "d2a": d2a, "d2b": d2b})

    res = run_bass_kernel_spmd(
        _program(), in_maps, list(range(NCORES)), trace=trace
    )
    out_g = np.empty((n, n), np.float32)
    for k in range(NCORES):
        i, h = divmod(k, 2)
        j0, j1 = 2 * h, 2 * h + 1
        out_g[i * BK : (i + 1) * BK, j0 * BK : (j0 + 1) * BK] = res.results[
            k
        ]["oa"].astype(np.float32)
        out_g[i * BK : (i + 1) * BK, j1 * BK : (j1 + 1) * BK] = res.results[
            k
        ]["ob"].astype(np.float32)
    out = np.empty((n, n), np.float32)
    out[np.ix_(perm, perm)] = out_g
    return out, res


def kernel(rho, thetas, A_stack, B_stack, C_stack, u_idx, p_idx):
    out, _ = _run(rho, thetas, A_stack, B_stack, C_stack, u_idx, p_idx)
    return out


# revision 15
# speedup vs baseline: 1.0472x; 1.0472x over previous
"""Trainium2 kernel for nn_Conv_RBS_density (496x496 density-matrix RBS circuit).

The reference applies 48 sequential RBS-gate conjugations
``rho <- U rho U^T`` where every ``U = cos(t)*A + sin(t)*B + C`` is an
orthogonal matrix of 30 disjoint 2x2 Givens rotations.  By associativity
the whole pipeline is ``out = V rho V^T`` with ``V = U48 @ ... @ U1``.
V is accumulated on the host with sparse Givens row updates
(O(48*30*N) flops — negligible); the O(N^3) work — the dense matmuls
against rho — runs on the 8 NeuronCores.

Structure: gates only couple basis states within connected components of
the qubit-tile graph, so V is block-diagonal (28 components of 16 states
+ 8 of 6).  Packing components into 4 bins of exactly 124 states gives a
grouped order where V_g = diag(B0..B3), each 124x124, and
``out_ij = B_i rho_ij B_j^T`` for the 16 124x124 blocks of the output.

Sharding: one (i, j-pair) per core — core k (i = k//2, h = k%2) computes
out blocks (i, 2h) and (i, 2h+1).  Per block: mm1 ``pk = rho_ij^T @ B_i^T``
(= (B_i rho_ij)^T, using rho's symmetry to avoid any transpose), then
mm2 ``out_ij = pk^T @ B_j^T``.  All operands bf16 (tolerance is 2e-2;
bf16 products accumulate in fp32 PSUM so only input rounding ~4e-3
matters) which makes matmuls 1-pass (4x fewer PE cycles than fp32) and
halves DMA bytes.  Only the two HWDGE queues (sync/scalar) are used —
SWDGE (gpsimd) adds ~3us of preamble memsets + postamble drain.

DMA layout: d1 (sync) = [B_i^T | rho_ij0] gates mm1_a as early as
possible; d2 (scalar) = [B_j0^T | rho_ij1 | B_j1^T] carries everything
else; outputs go out fp32 as two DMAs (scalar, then sync) as soon as
each block's PSUM->SBUF copy lands.  PE order mm1_a, mm1_b, mm2_a,
mm2_b keeps the PSUM copies off the critical path.
"""

import numpy as np

import concourse.mybir as mybir
from concourse import bacc
from concourse.bass_utils import run_bass_kernel_spmd
from concourse.tile import TileContext

N = 496          # C(32, 2) Hamming-weight-2 states
NCORES = 8
BK = 124         # packed block size
NB = N // BK     # 4 blocks

_cache = {}


def _build_program():
    nc = bacc.Bacc(
        "TRN2", target_bir_lowering=False, debug=False, num_devices=NCORES
    )
    f32 = mybir.dt.float32
    bf16 = mybir.dt.bfloat16
    d1_d = nc.dram_tensor("d1", [BK, 2 * BK], bf16, kind="ExternalInput")
    d2a_d = nc.dram_tensor("d2a", [BK, 2 * BK], bf16, kind="ExternalInput")
    d2b_d = nc.dram_tensor("d2b", [BK, BK], bf16, kind="ExternalInput")
    oa_d = nc.dram_tensor("oa", [BK, BK], bf16, kind="ExternalOutput")
    ob_d = nc.dram_tensor("ob", [BK, BK], bf16, kind="ExternalOutput")

    with TileContext(nc) as tc:
        with (
            tc.tile_pool(name="sbuf", bufs=1) as sbuf,
            tc.tile_pool(name="psum", bufs=1, space="PSUM") as psum,
        ):
            t1 = sbuf.tile([BK, 2 * BK], bf16, tag="t1", name="t1")
            t2a = sbuf.tile([BK, 2 * BK], bf16, tag="t2a", name="t2a")
            t2b = sbuf.tile([BK, BK], bf16, tag="t2b", name="t2b")
            # All three input DMAs ride the sync HWDGE ring in exact
            # need-order: both rings share one 4-SDMA-engine pool, so
            # splitting across rings only loses control of arrival
            # order, while single-queue FIFO lands d1 (gates both mm1s
            # via B_i^T) first with the full ~70 GB/s pool.
            nc.sync.dma_start(t1[:], d1_d[:, :])
            nc.sync.dma_start(t2a[:], d2a_d[:, :])
            nc.sync.dma_start(t2b[:], d2b_d[:, :])
            BiT = t1[:, 0:BK]
            rho_a = t1[:, BK : 2 * BK]
            Bj0T = t2a[:, 0:BK]
            rho_b = t2a[:, BK : 2 * BK]
            Bj1T = t2b[:, 0:BK]

            # mm1: pk = (B_i rho_ij)^T chunks, orientation ready for mm2 lhsT
            pka_ps = psum.tile([BK, BK], f32, tag="pka", name="pka")
            nc.tensor.matmul(pka_ps[:], rho_a, BiT, start=True, stop=True)
            pkb_ps = psum.tile([BK, BK], f32, tag="pkb", name="pkb")
            nc.tensor.matmul(pkb_ps[:], rho_b, BiT, start=True, stop=True)
            pka = sbuf.tile([BK, BK], bf16, tag="pka_sb", name="pka_sb")
            nc.vector.tensor_copy(pka[:], pka_ps[:])
            pkb = sbuf.tile([BK, BK], bf16, tag="pkb_sb", name="pkb_sb")
            nc.vector.tensor_copy(pkb[:], pkb_ps[:])

            # mm2: out_ij = pk^T @ B_j^T, DMA'd out as soon as copied
            oa_ps = psum.tile([BK, BK], f32, tag="oa", name="oa_ps")
            nc.tensor.matmul(oa_ps[:], pka[:], Bj0T, start=True, stop=True)
            oa_sb = sbuf.tile([BK, BK], bf16, tag="oa_sb", name="oa_sb")
            nc.scalar.copy(oa_sb[:], oa_ps[:])
            nc.scalar.dma_start(oa_d[:, :], oa_sb[:])

            ob_ps = psum.tile([BK, BK], f32, tag="ob", name="ob_ps")
            nc.tensor.matmul(ob_ps[:], pkb[:], Bj1T, start=True, stop=True)
            ob_sb = sbuf.tile([BK, BK], bf16, tag="ob_sb", name="ob_sb")
            nc.vector.tensor_copy(ob_sb[:], ob_ps[:])
            nc.sync.dma_start(ob_d[:, :], ob_sb[:])

    nc.compile()
    return nc


def _program():
    if "nc" not in _cache:
        _cache["nc"] = _build_program()
    return _cache["nc"]


def _gate_pairs(B_stack):
    """Per unique gate: (s, q) index arrays with B[u, s, q] = +1."""
    pairs = []
    for u in range(B_stack.shape[0]):
        pos = np.argwhere(B_stack[u] > 0.5)
        pairs.append((pos[:, 0], pos[:, 1]))
    return pairs


def _build_V(thetas, pairs, u_idx, p_idx, n):
    """V = U_G ... U_1 via sparse Givens row updates (float64)."""
    thetas = np.asarray(thetas, np.float64)
    cos_t, sin_t = np.cos(thetas), np.sin(thetas)
    V = np.eye(n)
    for g in range(len(u_idx)):
        u, p = int(u_idx[g]), int(p_idx[g])
        c, s = cos_t[p], sin_t[p]
        S, Q = pairs[u]
        vs, vq = V[S], V[Q]
        V[S] = c * vs + s * vq
        V[Q] = -s * vs + c * vq
    return V


def _grouping(pairs, n):
    """Union states coupled by any gate; pack components into NB bins of BK."""
    parent = list(range(n))

    def find(a):
        while parent[a] != a:
            parent[a] = parent[parent[a]]
            a = parent[a]
        return a

    for S, Q in pairs:
        for s, q in zip(S.tolist(), Q.tolist()):
            ra, rb = find(s), find(q)
            if ra != rb:
                parent[ra] = rb

    comps = {}
    for i in range(n):
        comps.setdefault(find(i), []).append(i)
    comps = sorted(comps.values(), key=len, reverse=True)

    bins = [[] for _ in range(NB)]
    for comp in comps:
        for b in bins:
            if len(b) + len(comp) <= BK:
                b.extend(comp)
                break
        else:
            raise ValueError("component packing failed")
    assert all(len(b) == BK for b in bins), [len(b) for b in bins]
    return np.array([i for b in bins for i in b], np.int64)


def _run(rho, thetas, A_stack, B_stack, C_stack, u_idx, p_idx, trace=False):
    import ml_dtypes

    bf16 = ml_dtypes.bfloat16
    rho = np.asarray(rho, np.float32)
    B_stack = np.asarray(B_stack)
    u_idx = np.asarray(u_idx).astype(np.int64)
    p_idx = np.asarray(p_idx).astype(np.int64)
    n = rho.shape[0]
    assert n == N, n

    if "struct" not in _cache:
        pairs = _gate_pairs(B_stack)
        _cache["struct"] = (pairs, _grouping(pairs, n))
    pairs, perm = _cache["struct"]

    V = _build_V(thetas, pairs, u_idx, p_idx, n).astype(np.float32)
    V_g = V[np.ix_(perm, perm)]
    rho_g = rho[np.ix_(perm, perm)]

    # block-diagonality check (structure is fixed by the module definition)
    blocks = [
        V_g[j * BK : (j + 1) * BK, j * BK : (j + 1) * BK] for j in range(NB)
    ]
    bd = np.zeros_like(V_g)
    for j in range(NB):
        bd[j * BK : (j + 1) * BK, j * BK : (j + 1) * BK] = blocks[j]
    assert np.array_equal(bd, V_g), "V lost block-diagonal structure"

    Bt = [np.ascontiguousarray(b.T).astype(bf16) for b in blocks]
    rho_bf = rho_g.astype(bf16)

    in_maps = []
    for k in range(NCORES):
        i, h = divmod(k, 2)
        j0, j1 = 2 * h, 2 * h + 1
        rr = rho_bf[i * BK : (i + 1) * BK]
        rho_a = rr[:, j0 * BK : (j0 + 1) * BK]
        rho_b = rr[:, j1 * BK : (j1 + 1) * BK]
        d1 = np.ascontiguousarray(np.concatenate([Bt[i], rho_a], axis=1))
        d2a = np.ascontiguousarray(np.concatenate([Bt[j0], rho_b], axis=1))
        d2b = np.ascontiguousarray(Bt[j1])
        in_maps.append({"d1": d1, "d2a": d2a, "d2b": d2b})

    res = run_bass_kernel_spmd(
        _program(), in_maps, list(range(NCORES)), trace=trace
    )
    out_g = np.empty((n, n), np.float32)
    for k in range(NCORES):
        i, h = divmod(k, 2)
        j0, j1 = 2 * h, 2 * h + 1
        out_g[i * BK : (i + 1) * BK, j0 * BK : (j0 + 1) * BK] = res.results[
            k
        ]["oa"].astype(np.float32)
        out_g[i * BK : (i + 1) * BK, j1 * BK : (j1 + 1) * BK] = res.results[
            k
        ]["ob"].astype(np.float32)
    out = np.empty((n, n), np.float32)
    out[np.ix_(perm, perm)] = out_g
    return out, res


def kernel(rho, thetas, A_stack, B_stack, C_stack, u_idx, p_idx):
    out, _ = _run(rho, thetas, A_stack, B_stack, C_stack, u_idx, p_idx)
    return out
